# revision 30
# baseline (speedup 1.0000x reference)
"""Multi-head causal attention with RoPE on 8 trn2 cores — v2.

Sharding: core c -> batch b = c // 4, head group g = c % 4 (heads 4g..4g+4).
Each core computes q/k/v projections for its 4 heads, causal attention, and
a partial output-projection (its heads' slice of Wo). The host sums the 4
partials per batch (tensor-parallel unshard) and adds the output bias.

v2 changes over the baseline (273us):
  - Input DMA is quarter-chunked and single-issue (3D access patterns), so
    the qk projection starts ~6us in instead of ~23us.
  - PV is col-tiled: per chunk, the 4 heads run as 2 psum banks x 2
    concurrent (0,0)/(0,64) array tiles; softmax denominators come from
    four concurrent M=1 ones-column matmuls into a third bank (partitions
    0/32/64/96) instead of a 65th v column.
  - The PV matmuls for chunk kk are emitted after the scores for chunk
    kk+1, so the softmax exp (ACT) overlaps the next chunk's scores and
    the PE never waits on the Scalar engine.
  - A fraction of the exp work runs on the Vector engine as a Schraudolph
    bf16 bit-trick (round(x*23.083 + 16251.4) -> int16, reinterpreted as
    bf16), offloading the ACT bottleneck.
  - The output projection of macro m is emitted inside macro m+1 and the
    result is written as fp16, so the out-proj tail disappears.
"""

import os

import numpy as np
import ml_dtypes

BF16 = ml_dtypes.bfloat16

B, S, F = 2, 2048, 1024
H, D = 16, 64
HALF = D // 2
NCORES = 8
HPC = 4  # heads per core
S_TILES = S // 128  # 16
N_CH = S // 512  # 4  (512-wide column chunks of s)
F_CH = F // 128  # 8
MACROS = 4  # q macro tiles of 512
MAX_WAVELENGTH = 10000.0

# Schraudolph bf16 exp: bits = round(x * (128*log2(e))/8 + (16256 - 128*c))
EXP_A = 184.6650558 / 8.0
EXP_B = 16256.0 - 128.0 * 0.0362

_CACHE = {}
LAST_RESULT = None


def _build_nc():
    import concourse.bacc as bacc
    import concourse.tile as tile
    import concourse.mybir as mybir
    import concourse.bass as bass

    fp32 = mybir.dt.float32
    fp16 = mybir.dt.float16
    bf16 = mybir.dt.bfloat16
    i16 = mybir.dt.int16
    MULT = mybir.AluOpType.mult
    ADD = mybir.AluOpType.add
    EXP = mybir.ActivationFunctionType.Exp
    IDENT = mybir.ActivationFunctionType.Identity

    exp_mode = os.environ.get("KEXP", "mix50")  # act | mix25 | mix50 | dve
    mask_eng = os.environ.get("KMASK", "gps")  # gps | dve
    kdbg = os.environ.get("KDBG", "0") == "1"

    nc = bacc.Bacc("TRN2", target_bir_lowering=False, debug=False)

    # x transposed, quarter-chunk-major: block (n, kc) = x[b].T[128kc:.., 512n:..]
    xT_d = nc.dram_tensor("xT", [N_CH * F_CH * 128, 512], bf16, kind="ExternalInput")
    wq_d = nc.dram_tensor("wq", [F, 256], bf16, kind="ExternalInput")
    wk_d = nc.dram_tensor("wk", [F, 256], bf16, kind="ExternalInput")
    wv_d = nc.dram_tensor("wv", [F + 1, 256], bf16, kind="ExternalInput")
    wo_d = nc.dram_tensor("wo", [256, F], bf16, kind="ExternalInput")
    bqk_d = nc.dram_tensor("bqk", [128, 4], fp32, kind="ExternalInput")
    sel4_d = nc.dram_tensor("sel4", [4, 256], fp32, kind="ExternalInput")
    cos_d = nc.dram_tensor("cosw", [128, S], bf16, kind="ExternalInput")
    sin_d = nc.dram_tensor("sinw", [128, S], bf16, kind="ExternalInput")
    mask_d = nc.dram_tensor("mask", [128, 256], bf16, kind="ExternalInput")
    outT_d = nc.dram_tensor("outT", [F, S], fp16, kind="ExternalOutput")
    if kdbg:
        dbg_ao_d = [nc.dram_tensor(f"dbg_ao{i}", [128, S], bf16, kind="ExternalOutput") for i in range(2)]
        dbg_wo_d = nc.dram_tensor("dbg_wo", [128, 2 * F], bf16, kind="ExternalOutput")
        dbg_den_d = nc.dram_tensor("dbg_den", [4, 4 * 512], fp32, kind="ExternalOutput")

    with tile.TileContext(nc) as tc:
        with (
            tc.tile_pool(name="persist", bufs=1) as persist,
            tc.tile_pool(name="tmp", bufs=8) as tmp,
            tc.tile_pool(name="attn", bufs=8) as attn_pool,
            tc.tile_pool(name="norm", bufs=4) as norm_pool,
            tc.tile_pool(name="ostage", bufs=8) as ostage,
            tc.tile_pool(name="psA", bufs=2, space="PSUM") as psA,
            tc.tile_pool(name="psPV", bufs=2, space="PSUM") as psPV,
            tc.tile_pool(name="psD", bufs=2, space="PSUM") as psD,
        ):
            # ---------------- persistent SBUF + single-issue loads ----------
            xT_all = persist.tile([128, F_CH * S], bf16, tag="xT", name="xT")
            xT = [xT_all[:, S * kc : S * (kc + 1)] for kc in range(F_CH)]
            xones = persist.tile([1, S], bf16, tag="xones", name="xones")
            nc.vector.memset(xones[:, :], 1.0)

            wq_all = persist.tile([128, F_CH * 256], bf16, tag="wq", name="wq")
            wk_all = persist.tile([128, F_CH * 256], bf16, tag="wk", name="wk")
            wv_all = persist.tile([128, F_CH * 256], bf16, tag="wv", name="wv")
            wq = [wq_all[:, 256 * i : 256 * (i + 1)] for i in range(F_CH)]
            wk = [wk_all[:, 256 * i : 256 * (i + 1)] for i in range(F_CH)]
            wv = [wv_all[:, 256 * i : 256 * (i + 1)] for i in range(F_CH)]
            wvb = persist.tile([1, 256], bf16, tag="wvb", name="wvb")
            wo_all = persist.tile([128, 2 * F], bf16, tag="wo", name="wo")
            wo = [wo_all[:, F * i : F * (i + 1)] for i in range(2)]

            bqk = persist.tile([128, 4], fp32, tag="bqk", name="bqk")
            cosw = persist.tile([128, S], bf16, tag="cosw", name="cosw")
            sinw = persist.tile([128, S], bf16, tag="sinw", name="sinw")
            maskt = persist.tile([128, 256], bf16, tag="maskt", name="maskt")

            # one dma_start per tensor (3D APs), issued in first-use order
            wq_v = wq_all[:, :].rearrange("p (c w) -> p c w", c=F_CH)
            wk_v = wk_all[:, :].rearrange("p (c w) -> p c w", c=F_CH)
            wv_v = wv_all[:, :].rearrange("p (c w) -> p c w", c=F_CH)
            wq_s = wq_d[:, :].rearrange("(c p) w -> p c w", p=128)
            wk_s = wk_d[:, :].rearrange("(c p) w -> p c w", p=128)
            wv_s = wv_d[0:F, :].rearrange("(c p) w -> p c w", p=128)

            def x_quarter_dma(n, splits=1):
                xv4 = xT_all[:, :].rearrange("p (c q w) -> p c q w", c=F_CH, q=N_CH)
                step = F_CH // splits
                for s in range(splits):
                    csl = slice(step * s, step * (s + 1))
                    xv = xv4[:, csl, n, :]
                    xs = xT_d[128 * (F_CH * n + step * s) : 128 * (F_CH * n + step * (s + 1)), :]
                    xs = xs.rearrange("(c p) w -> p c w", p=128)
                    nc.sync.dma_start(out=xv, in_=xs)

            wq_v4 = wq_all[:, :].rearrange("p (c w) -> p c w", c=F_CH)
            nc.sync.dma_start(out=wq_v4[:, 0:4, :], in_=wq_d[0:512, :].rearrange("(c p) w -> p c w", p=128))
            nc.sync.dma_start(out=wq_v4[:, 4:8, :], in_=wq_d[512:1024, :].rearrange("(c p) w -> p c w", p=128))
            x_quarter_dma(0, splits=2)
            nc.sync.dma_start(out=wk_v, in_=wk_s)
            sel4 = persist.tile([4, 256], fp32, tag="sel4", name="sel4")
            nc.sync.dma_start(out=bqk, in_=bqk_d[:, :])
            nc.sync.dma_start(out=sel4, in_=sel4_d[:, :])
            nc.sync.dma_start(out=wv_v, in_=wv_s)
            nc.sync.dma_start(out=wvb, in_=wv_d[F : F + 1, :])
            nc.sync.dma_start(out=cosw, in_=cos_d[:, :])
            nc.sync.dma_start(out=sinw, in_=sin_d[:, :])
            nc.sync.dma_start(out=maskt, in_=mask_d[:, :])
            x_quarter_dma(1)
            x_quarter_dma(2)
            x_quarter_dma(3)
            wo_v = wo_all[:, :].rearrange("p (c w) -> p c w", c=2)
            wo_s = wo_d[:, :].rearrange("(c p) w -> p c w", p=128)
            nc.sync.dma_start(out=wo_v, in_=wo_s)

            ones1 = persist.tile([128, 1], bf16, tag="ones1", name="ones1")
            nc.vector.memset(ones1[:, :], 1.0)

            # post-RoPE q/k, transposed layout [d, s]; chunk 0 = x1 halves
            # of the 4 heads (head h -> partitions 32h..32h+32), chunk 1 = x2.
            q1 = persist.tile([128, S], bf16, tag="q1", name="q1")
            q2 = persist.tile([128, S], bf16, tag="q2", name="q2")
            k1 = persist.tile([128, S], bf16, tag="k1", name="k1")
            k2 = persist.tile([128, S], bf16, tag="k2", name="k2")
            # v in [s, d] layout; head h at cols 64h..64h+64
            v_sb = [persist.tile([128, 256], bf16, tag=f"v{i}", name=f"v{i}") for i in range(S_TILES)]
            # attention output, [dh, s]; pair p -> tile p (head 2p rows 0:64, 2p+1 rows 64:128)
            aoT = [persist.tile([128, S], bf16, tag=f"aoT{i}", name=f"aoT{i}") for i in range(2)]

            # ---------------- q/k projection + RoPE for one 512-chunk -------
            def qk_chunk(n):
                nsl = slice(512 * n, 512 * (n + 1))
                for (w_sb, b0, o1, o2) in ((wq, 0, q1, q2), (wk, 2, k1, k2)):
                    ps1 = psA.tile([128, 512], fp32, tag="sc", name="ps1")
                    ps2 = psA.tile([128, 512], fp32, tag="sc", name="ps2")
                    for kc in range(F_CH):
                        nc.tensor.matmul(ps1, w_sb[kc][:, 0:128], xT[kc][:, nsl],
                                         start=(kc == 0), stop=(kc == F_CH - 1))
                    for kc in range(F_CH):
                        nc.tensor.matmul(ps2, w_sb[kc][:, 128:256], xT[kc][:, nsl],
                                         start=(kc == 0), stop=(kc == F_CH - 1))
                    c1 = tmp.tile([128, 512], bf16, tag="rope", name="c1")
                    c2 = tmp.tile([128, 512], bf16, tag="rope", name="c2")
                    nc.scalar.activation(c1, ps1, func=IDENT, bias=bqk[:, b0:b0 + 1])
                    nc.scalar.activation(c2, ps2, func=IDENT, bias=bqk[:, b0 + 1:b0 + 2])
                    t1 = tmp.tile([128, 512], bf16, tag="rope", name="t1")
                    t2 = tmp.tile([128, 512], bf16, tag="rope", name="t2")
                    t3 = tmp.tile([128, 512], bf16, tag="rope", name="t3")
                    t4 = tmp.tile([128, 512], bf16, tag="rope", name="t4")
                    nc.vector.tensor_mul(t1, c1, cosw[:, nsl])
                    nc.vector.tensor_mul(t2, c2, sinw[:, nsl])
                    nc.vector.tensor_mul(t3, c2, cosw[:, nsl])
                    nc.vector.tensor_mul(t4, c1, sinw[:, nsl])
                    nc.vector.tensor_sub(o1[:, nsl], t1, t2)
                    nc.vector.tensor_add(o2[:, nsl], t3, t4)

            def v_chunk(st):
                ps = psA.tile([128, 256], fp32, tag="sc", name="psv")
                sl = slice(128 * st, 128 * (st + 1))
                for kc in range(F_CH):
                    nc.tensor.matmul(ps, xT[kc][:, sl], wv[kc], start=(kc == 0), stop=False)
                nc.tensor.matmul(ps, xones[:, sl], wvb, start=False, stop=True)
                nc.scalar.copy(v_sb[st], ps)

            qk_chunk(0)
            for st in range(4):
                v_chunk(st)
            qk_chunk(1)
            for st in range(4, 8):
                v_chunk(st)
            qk_chunk(2)
            qk_chunk(3)
            for st in range(8, 16):
                v_chunk(st)

            # ---------------- attention -------------------------------------
            # scores transposed [kk, q]; pair p's two heads share a [128,1024]
            # psum tile. exp -> at (int16 tile, bf16 bit view). PV lags one
            # chunk: pair psum [128,512] = head 2p rows 0:64 (tile col 0),
            # head 2p+1 rows 64:128 (tile col 64); denominators = 4 concurrent
            # M=1 ones matmuls into psD partitions 0/32/64/96.
            exp_ctr = [0]

            def emit_scores_exp(m, kk):
                t = kk - 4 * m
                lo = max(0, t) * 128
                ksl = slice(128 * kk, 128 * (kk + 1))
                ats = []
                for p in range(2):
                    sps = psA.tile([128, 1024], fp32, tag="sc", name="sps")
                    for hh in range(2):
                        h = 2 * p + hh
                        hp = slice(32 * h, 32 * (h + 1))
                        tp = (32 * h, 0)
                        qsl = slice(512 * m + lo, 512 * (m + 1))
                        osl = slice(512 * hh + lo, 512 * hh + 512)
                        nc.tensor.matmul(sps[:, osl], k1[hp, ksl], q1[hp, qsl],
                                         start=True, stop=False, tile_position=tp)
                        nc.tensor.matmul(sps[:, osl], k2[hp, ksl], q2[hp, qsl],
                                         start=False, stop=True, tile_position=tp)
                    at = attn_pool.tile([128, 1024], i16, tag="attn", name="at")
                    at_bf = at[:, :].bitcast(bf16)
                    sps_v = sps[:, :].rearrange("a (h q) -> a h q", h=2)[:, :, lo:512]
                    at_v = at_bf.rearrange("a (h q) -> a h q", h=2)[:, :, lo:512]
                    use_dve = (
                        exp_mode == "dve"
                        or (exp_mode == "mix50" and exp_ctr[0] % 2 == 1)
                        or (exp_mode == "mix25" and exp_ctr[0] % 4 == 3)
                    )
                    exp_ctr[0] += 1
                    if use_dve:
                        at_i = at[:, :].rearrange("a (h q) -> a h q", h=2)[:, :, lo:512]
                        nc.vector.tensor_scalar(out=at_i, in0=sps_v,
                                                scalar1=EXP_A, scalar2=EXP_B,
                                                op0=MULT, op1=ADD)
                    else:
                        nc.scalar.activation(out=at_v, in_=sps_v, func=EXP, scale=0.125)
                    if t >= 0:
                        dv = at_bf.rearrange("a (h q) -> a h q", h=2)[:, :, 128 * t:128 * (t + 1)]
                        mv = maskt[:, :].rearrange("a (h q) -> a h q", h=2)
                        if mask_eng == "gps":
                            nc.gpsimd.tensor_tensor(dv, dv, mv, op=MULT)
                        else:
                            nc.vector.tensor_tensor(dv, dv, mv, op=MULT)
                    ats.append(at_bf)
                return ats, lo

            def emit_pv(m, kk, ats, lo, pv_ps, den_ps, last):
                for p in range(2):
                    for hh in range(2):
                        h = 2 * p + hh
                        nc.tensor.matmul(
                            pv_ps[p][64 * hh:64 * hh + 64, lo:512],
                            v_sb[kk][:, 64 * h:64 * h + 64],
                            ats[p][:, 512 * hh + lo:512 * hh + 512],
                            start=(kk == 0), stop=last)
                for h in range(HPC):
                    p, hh = h // 2, h % 2
                    nc.tensor.matmul(
                        den_ps[32 * h:32 * h + 1, lo:512],
                        ones1[:, 0:1],
                        ats[p][:, 512 * hh + lo:512 * hh + 512],
                        start=(kk == 0), stop=last,
                        tile_position=(0, 32 * h))

            def emit_normalize_a(m, pv_ps, den_ps):
                # PE-free part, emitted right at the macro boundary: free the
                # pv/den banks quickly, gather denominator rows, reciprocal.
                # DMA cannot read PSUM and engine APs cap base partition at
                # 64, so: engine-copy out, strided-DMA the scattered rows to
                # partition 0, reciprocal there.
                sp = [norm_pool.tile([128, 512], fp32, tag="sp", name="sp") for _ in range(2)]
                for p in range(2):
                    nc.vector.tensor_copy(sp[p], pv_ps[p])
                den_sb = norm_pool.tile([128, 512], fp32, tag="den_sb", name="den_sb")
                nc.scalar.copy(den_sb, den_ps)
                den4 = norm_pool.tile([4, 512], fp32, tag="den4", name="den4")
                den_v = den_sb[:, :].rearrange("(a b) w -> a b w", b=32)[:, 0, :]
                nc.sync.dma_start(out=den4, in_=den_v)
                if kdbg:
                    nc.sync.dma_start(out=dbg_den_d[:, 512 * m:512 * (m + 1)], in_=den4)
                rcp4 = norm_pool.tile([4, 512], fp32, tag="rcp4", name="rcp4")
                nc.vector.reciprocal_approx_fast(rcp4, den4)
                return sp, rcp4

            def emit_normalize_b(m, sp, rcp4):
                # PE part, deferred into the next macro's chunk stream so the
                # reciprocal chain latency never idles the PE at a boundary.
                # Broadcast reciprocal row h down 64 partitions with a K=4
                # selector matmul (sel4 col block 64h has row h all-ones);
                # partition_broadcast cannot write at a base-64 output.
                msl = slice(512 * m, 512 * (m + 1))
                for p in range(2):
                    rb_ps = psD.tile([128, 512], fp32, tag="rb", bufs=1, name="rb_ps")
                    nc.tensor.matmul(rb_ps[0:64, :], sel4[:, 64 * (2 * p):64 * (2 * p) + 64],
                                     rcp4, start=True, stop=True, tile_position=(0, 0))
                    nc.tensor.matmul(rb_ps[64:128, :], sel4[:, 64 * (2 * p + 1):64 * (2 * p + 1) + 64],
                                     rcp4, start=True, stop=True, tile_position=(0, 64))
                    nc.vector.tensor_tensor(aoT[p][:, msl], sp[p], rb_ps, op=MULT)

            def emit_outproj(m):
                msl = slice(512 * m, 512 * (m + 1))
                for fo in range(F_CH):
                    fsl = slice(128 * fo, 128 * (fo + 1))
                    pw = psA.tile([128, 512], fp32, tag="sc", name="pw")
                    for c in range(2):
                        nc.tensor.matmul(pw, wo[c][:, fsl], aoT[c][:, msl],
                                         start=(c == 0), stop=(c == 1))
                    ow = ostage.tile([128, 512], fp16, tag="ow", name="ow")
                    if fo % 2 == 0:
                        nc.vector.tensor_copy(ow, pw)
                    else:
                        nc.scalar.copy(ow, pw)
                    nc.sync.dma_start(out=outT_d[fsl, msl], in_=ow)

            prev_norm = None
            for m in range(MACROS):
                nkk = 4 * m + 4
                pv_ps = [psPV.tile([128, 512], fp32, tag="pvp", name="pvp") for _ in range(2)]
                den_ps = psD.tile([128, 512], fp32, tag="den", bufs=1, name="den")
                pending = []
                for kk in range(nkk):
                    ats, lo = emit_scores_exp(m, kk)
                    if len(pending) == 2:
                        pk, pats, plo = pending.pop(0)
                        emit_pv(m, pk, pats, plo, pv_ps, den_ps, last=False)
                    pending.append((kk, ats, lo))
                    if kk == 1 and prev_norm is not None:
                        emit_normalize_b(m - 1, *prev_norm)
                        prev_norm = None
                    if kk == nkk - 2 and m > 0:
                        emit_outproj(m - 1)
                for i, (pk, pats, plo) in enumerate(pending):
                    emit_pv(m, pk, pats, plo, pv_ps, den_ps, last=(i == len(pending) - 1))
                prev_norm = emit_normalize_a(m, pv_ps, den_ps)
            emit_normalize_b(MACROS - 1, *prev_norm)
            emit_outproj(MACROS - 1)
            if kdbg:
                for i in range(2):
                    nc.sync.dma_start(out=dbg_ao_d[i][:, :], in_=aoT[i])
                nc.sync.dma_start(out=dbg_wo_d[:, :], in_=wo_all)

    nc.compile()
    return nc


def _get_nc():
    if "nc" not in _CACHE:
        _CACHE["nc"] = _build_nc()
    return _CACHE["nc"]


def _host_prep(x, positions, Wq, bq, Wk, bk, Wv, bv, Wo, bo):
    """Build the 8 per-core input maps."""
    ts = MAX_WAVELENGTH ** (2.0 * np.arange(HALF, dtype=np.float32) / D)  # [32]
    in_maps = []
    for c in range(NCORES):
        b, g = c // 4, c % 4
        heads = np.arange(4 * g, 4 * g + 4)
        cols_x1 = np.concatenate([64 * h + np.arange(32) for h in heads])
        cols_x2 = cols_x1 + 32
        perm = np.concatenate([cols_x1, cols_x2])

        # x^T quarter-chunk-major: block (n, kc) contiguous
        xTb = np.ascontiguousarray(x[b].T.astype(BF16))  # [F, S]
        xT = np.ascontiguousarray(
            xTb.reshape(F_CH, 128, N_CH, 512).transpose(2, 0, 1, 3)
        ).reshape(N_CH * F_CH * 128, 512)

        wv_e = np.zeros((F + 1, 256), dtype=np.float32)
        for hl, h in enumerate(heads):
            wv_e[:F, 64 * hl:64 * hl + 64] = Wv[:, 64 * h:64 * h + 64]
            wv_e[F, 64 * hl:64 * hl + 64] = bv[64 * h:64 * h + 64]

        bqk = np.stack([bq[cols_x1], bq[cols_x2], bk[cols_x1], bk[cols_x2]],
                       axis=1).astype(np.float32)  # [128, 4]

        pos = positions[b].astype(np.float32)  # [S]
        ang = pos[None, :] / ts[:, None]  # [32, S]
        cosw = np.tile(np.cos(ang), (4, 1)).astype(BF16)
        sinw = np.tile(np.sin(ang), (4, 1)).astype(BF16)

        ii = np.arange(128)
        mask = np.tile((ii[:, None] <= ii[None, :]).astype(BF16), (1, 2))

        sel4 = np.zeros((4, 256), dtype=np.float32)
        for h in range(4):
            sel4[h, 64 * h:64 * h + 64] = 1.0

        in_maps.append({
            "xT": xT,
            "wq": Wq[:, perm].astype(BF16),
            "wk": Wk[:, perm].astype(BF16),
            "wv": wv_e.astype(BF16),
            "wo": Wo[64 * heads[0]:64 * heads[0] + 256, :].astype(BF16),
            "bqk": bqk,
            "sel4": sel4,
            "cosw": cosw,
            "sinw": sinw,
            "mask": np.ascontiguousarray(mask),
        })
    return in_maps


def kernel(x, positions, Wq, bq, Wk, bk, Wv, bv, Wo, bo):
    global LAST_RESULT
    from concourse.bass_utils import run_bass_kernel_spmd

    x = np.asarray(x, dtype=np.float32)
    positions = np.asarray(positions)
    args = [np.asarray(a, dtype=np.float32) for a in (Wq, bq, Wk, bk, Wv, bv, Wo, bo)]
    Wq, bq, Wk, bk, Wv, bv, Wo, bo = args

    nc = _get_nc()
    in_maps = _host_prep(x, positions, Wq, bq, Wk, bk, Wv, bv, Wo, bo)
    try:
        res = run_bass_kernel_spmd(nc, in_maps, core_ids=list(range(NCORES)))
    except ModuleNotFoundError:
        # axon NTFF profiling hook unavailable in this image; run untraced
        os.environ["BASS_NEVER_TRACE"] = "1"
        res = run_bass_kernel_spmd(nc, in_maps, core_ids=list(range(NCORES)))
    LAST_RESULT = res

    out = np.empty((B, S, F), dtype=np.float32)
    for b in range(B):
        acc = np.zeros((F, S), dtype=np.float32)
        for g in range(4):
            acc += res.results[4 * b + g]["outT"].astype(np.float32)
        out[b] = acc.T + bo[None, :]
    return out


# revision 32
# speedup vs baseline: 1.0189x; 1.0189x over previous
"""Multi-head causal attention with RoPE on 8 trn2 cores — v2.

Sharding: core c -> batch b = c // 4, head group g = c % 4 (heads 4g..4g+4).
Each core computes q/k/v projections for its 4 heads, causal attention, and
a partial output-projection (its heads' slice of Wo). The host sums the 4
partials per batch (tensor-parallel unshard) and adds the output bias.

v2 changes over the baseline (273us):
  - Input DMA is quarter-chunked and single-issue (3D access patterns), so
    the qk projection starts ~6us in instead of ~23us.
  - PV is col-tiled: per chunk, the 4 heads run as 2 psum banks x 2
    concurrent (0,0)/(0,64) array tiles; softmax denominators come from
    four concurrent M=1 ones-column matmuls into a third bank (partitions
    0/32/64/96) instead of a 65th v column.
  - The PV matmuls for chunk kk are emitted after the scores for chunk
    kk+1, so the softmax exp (ACT) overlaps the next chunk's scores and
    the PE never waits on the Scalar engine.
  - A fraction of the exp work runs on the Vector engine as a Schraudolph
    bf16 bit-trick (round(x*23.083 + 16251.4) -> int16, reinterpreted as
    bf16), offloading the ACT bottleneck.
  - The output projection of macro m is emitted inside macro m+1 and the
    result is written as fp16, so the out-proj tail disappears.
"""

import os

import numpy as np
import ml_dtypes

BF16 = ml_dtypes.bfloat16

B, S, F = 2, 2048, 1024
H, D = 16, 64
HALF = D // 2
NCORES = 8
HPC = 4  # heads per core
S_TILES = S // 128  # 16
N_CH = S // 512  # 4  (512-wide column chunks of s)
F_CH = F // 128  # 8
MACROS = 4  # q macro tiles of 512
MAX_WAVELENGTH = 10000.0

# Schraudolph bf16 exp: bits = round(x * (128*log2(e))/8 + (16256 - 128*c))
EXP_A = 184.6650558 / 8.0
EXP_B = 16256.0 - 128.0 * 0.0362

_CACHE = {}
LAST_RESULT = None


def _build_nc():
    import concourse.bacc as bacc
    import concourse.tile as tile
    import concourse.mybir as mybir
    import concourse.bass as bass

    fp32 = mybir.dt.float32
    fp16 = mybir.dt.float16
    bf16 = mybir.dt.bfloat16
    i16 = mybir.dt.int16
    MULT = mybir.AluOpType.mult
    ADD = mybir.AluOpType.add
    EXP = mybir.ActivationFunctionType.Exp
    IDENT = mybir.ActivationFunctionType.Identity

    exp_mode = os.environ.get("KEXP", "mix50")  # act | mix25 | mix50 | dve
    mask_eng = os.environ.get("KMASK", "gps")  # gps | dve
    kdbg = os.environ.get("KDBG", "0") == "1"

    nc = bacc.Bacc("TRN2", target_bir_lowering=False, debug=False)

    # x transposed, quarter-chunk-major: block (n, kc) = x[b].T[128kc:.., 512n:..]
    xT_d = nc.dram_tensor("xT", [N_CH * F_CH * 128, 512], bf16, kind="ExternalInput")
    wq_d = nc.dram_tensor("wq", [F, 256], bf16, kind="ExternalInput")
    wk_d = nc.dram_tensor("wk", [F, 256], bf16, kind="ExternalInput")
    wv_d = nc.dram_tensor("wv", [F + 1, 256], bf16, kind="ExternalInput")
    wo_d = nc.dram_tensor("wo", [256, F], bf16, kind="ExternalInput")
    bqk_d = nc.dram_tensor("bqk", [128, 4], fp32, kind="ExternalInput")
    sel4_d = nc.dram_tensor("sel4", [4, 256], fp32, kind="ExternalInput")
    cos_d = nc.dram_tensor("cosw", [128, S], bf16, kind="ExternalInput")
    sin_d = nc.dram_tensor("sinw", [128, S], bf16, kind="ExternalInput")
    mask_d = nc.dram_tensor("mask", [128, 256], bf16, kind="ExternalInput")
    outT_d = nc.dram_tensor("outT", [F, S], fp16, kind="ExternalOutput")
    if kdbg:
        dbg_ao_d = [nc.dram_tensor(f"dbg_ao{i}", [128, S], bf16, kind="ExternalOutput") for i in range(2)]
        dbg_wo_d = nc.dram_tensor("dbg_wo", [128, 2 * F], bf16, kind="ExternalOutput")
        dbg_den_d = nc.dram_tensor("dbg_den", [4, 4 * 512], fp32, kind="ExternalOutput")

    with tile.TileContext(nc) as tc:
        with (
            tc.tile_pool(name="persist", bufs=1) as persist,
            tc.tile_pool(name="tmp", bufs=8) as tmp,
            tc.tile_pool(name="attn", bufs=8) as attn_pool,
            tc.tile_pool(name="norm", bufs=4) as norm_pool,
            tc.tile_pool(name="ostage", bufs=8) as ostage,
            tc.tile_pool(name="psA", bufs=2, space="PSUM") as psA,
            tc.tile_pool(name="psPV", bufs=2, space="PSUM") as psPV,
            tc.tile_pool(name="psD", bufs=2, space="PSUM") as psD,
        ):
            # ---------------- persistent SBUF + single-issue loads ----------
            xT_all = persist.tile([128, F_CH * S], bf16, tag="xT", name="xT")
            xT = [xT_all[:, S * kc : S * (kc + 1)] for kc in range(F_CH)]
            xones = persist.tile([1, S], bf16, tag="xones", name="xones")
            nc.vector.memset(xones[:, :], 1.0)

            wq_all = persist.tile([128, F_CH * 256], bf16, tag="wq", name="wq")
            wk_all = persist.tile([128, F_CH * 256], bf16, tag="wk", name="wk")
            wv_all = persist.tile([128, F_CH * 256], bf16, tag="wv", name="wv")
            wq = [wq_all[:, 256 * i : 256 * (i + 1)] for i in range(F_CH)]
            wk = [wk_all[:, 256 * i : 256 * (i + 1)] for i in range(F_CH)]
            wv = [wv_all[:, 256 * i : 256 * (i + 1)] for i in range(F_CH)]
            wvb = persist.tile([1, 256], bf16, tag="wvb", name="wvb")
            wo_all = persist.tile([128, 2 * F], bf16, tag="wo", name="wo")
            wo = [wo_all[:, F * i : F * (i + 1)] for i in range(2)]

            bqk = persist.tile([128, 4], fp32, tag="bqk", name="bqk")
            cosw = persist.tile([128, S], bf16, tag="cosw", name="cosw")
            sinw = persist.tile([128, S], bf16, tag="sinw", name="sinw")
            maskt = persist.tile([128, 256], bf16, tag="maskt", name="maskt")

            # one dma_start per tensor (3D APs), issued in first-use order
            wq_v = wq_all[:, :].rearrange("p (c w) -> p c w", c=F_CH)
            wk_v = wk_all[:, :].rearrange("p (c w) -> p c w", c=F_CH)
            wv_v = wv_all[:, :].rearrange("p (c w) -> p c w", c=F_CH)
            wq_s = wq_d[:, :].rearrange("(c p) w -> p c w", p=128)
            wk_s = wk_d[:, :].rearrange("(c p) w -> p c w", p=128)
            wv_s = wv_d[0:F, :].rearrange("(c p) w -> p c w", p=128)

            def x_quarter_dma(n, splits=1):
                xv4 = xT_all[:, :].rearrange("p (c q w) -> p c q w", c=F_CH, q=N_CH)
                step = F_CH // splits
                for s in range(splits):
                    csl = slice(step * s, step * (s + 1))
                    xv = xv4[:, csl, n, :]
                    xs = xT_d[128 * (F_CH * n + step * s) : 128 * (F_CH * n + step * (s + 1)), :]
                    xs = xs.rearrange("(c p) w -> p c w", p=128)
                    nc.sync.dma_start(out=xv, in_=xs)

            wq_v4 = wq_all[:, :].rearrange("p (c w) -> p c w", c=F_CH)
            nc.sync.dma_start(out=wq_v4[:, 0:4, :], in_=wq_d[0:512, :].rearrange("(c p) w -> p c w", p=128))
            nc.sync.dma_start(out=wq_v4[:, 4:8, :], in_=wq_d[512:1024, :].rearrange("(c p) w -> p c w", p=128))
            x_quarter_dma(0, splits=2)
            nc.sync.dma_start(out=wk_v, in_=wk_s)
            sel4 = persist.tile([4, 256], fp32, tag="sel4", name="sel4")
            nc.sync.dma_start(out=bqk, in_=bqk_d[:, :])
            nc.sync.dma_start(out=sel4, in_=sel4_d[:, :])
            nc.sync.dma_start(out=wv_v, in_=wv_s)
            nc.sync.dma_start(out=wvb, in_=wv_d[F : F + 1, :])
            nc.sync.dma_start(out=cosw, in_=cos_d[:, :])
            nc.sync.dma_start(out=sinw, in_=sin_d[:, :])
            nc.sync.dma_start(out=maskt, in_=mask_d[:, :])
            x_quarter_dma(1)
            x_quarter_dma(2)
            x_quarter_dma(3)
            wo_v = wo_all[:, :].rearrange("p (c w) -> p c w", c=2)
            wo_s = wo_d[:, :].rearrange("(c p) w -> p c w", p=128)
            nc.sync.dma_start(out=wo_v, in_=wo_s)

            ones1 = persist.tile([128, 1], bf16, tag="ones1", name="ones1")
            nc.vector.memset(ones1[:, :], 1.0)

            # post-RoPE q/k, transposed layout [d, s]; chunk 0 = x1 halves
            # of the 4 heads (head h -> partitions 32h..32h+32), chunk 1 = x2.
            q1 = persist.tile([128, S], bf16, tag="q1", name="q1")
            q2 = persist.tile([128, S], bf16, tag="q2", name="q2")
            k1 = persist.tile([128, S], bf16, tag="k1", name="k1")
            k2 = persist.tile([128, S], bf16, tag="k2", name="k2")
            # v in [s, d] layout; head h at cols 64h..64h+64
            v_sb = [persist.tile([128, 256], bf16, tag=f"v{i}", name=f"v{i}") for i in range(S_TILES)]
            # attention output, [dh, s]; pair p -> tile p (head 2p rows 0:64, 2p+1 rows 64:128)
            aoT = [persist.tile([128, S], bf16, tag=f"aoT{i}", name=f"aoT{i}") for i in range(2)]

            # ---------------- q/k projection + RoPE for one 512-chunk -------
            def qk_chunk(n):
                nsl = slice(512 * n, 512 * (n + 1))
                for (w_sb, b0, o1, o2) in ((wq, 0, q1, q2), (wk, 2, k1, k2)):
                    ps1 = psA.tile([128, 512], fp32, tag="sc", name="ps1")
                    ps2 = psA.tile([128, 512], fp32, tag="sc", name="ps2")
                    for kc in range(F_CH):
                        nc.tensor.matmul(ps1, w_sb[kc][:, 0:128], xT[kc][:, nsl],
                                         start=(kc == 0), stop=(kc == F_CH - 1))
                    for kc in range(F_CH):
                        nc.tensor.matmul(ps2, w_sb[kc][:, 128:256], xT[kc][:, nsl],
                                         start=(kc == 0), stop=(kc == F_CH - 1))
                    c1 = tmp.tile([128, 512], bf16, tag="rope", name="c1")
                    c2 = tmp.tile([128, 512], bf16, tag="rope", name="c2")
                    nc.scalar.activation(c1, ps1, func=IDENT, bias=bqk[:, b0:b0 + 1])
                    nc.scalar.activation(c2, ps2, func=IDENT, bias=bqk[:, b0 + 1:b0 + 2])
                    t1 = tmp.tile([128, 512], bf16, tag="rope", name="t1")
                    t2 = tmp.tile([128, 512], bf16, tag="rope", name="t2")
                    t3 = tmp.tile([128, 512], bf16, tag="rope", name="t3")
                    t4 = tmp.tile([128, 512], bf16, tag="rope", name="t4")
                    nc.vector.tensor_mul(t1, c1, cosw[:, nsl])
                    nc.vector.tensor_mul(t2, c2, sinw[:, nsl])
                    nc.vector.tensor_mul(t3, c2, cosw[:, nsl])
                    nc.vector.tensor_mul(t4, c1, sinw[:, nsl])
                    nc.vector.tensor_sub(o1[:, nsl], t1, t2)
                    nc.vector.tensor_add(o2[:, nsl], t3, t4)

            def v_chunk(st):
                ps = psA.tile([128, 256], fp32, tag="sc", name="psv")
                sl = slice(128 * st, 128 * (st + 1))
                for kc in range(F_CH):
                    nc.tensor.matmul(ps, xT[kc][:, sl], wv[kc], start=(kc == 0), stop=False)
                nc.tensor.matmul(ps, xones[:, sl], wvb, start=False, stop=True)
                nc.scalar.copy(v_sb[st], ps)

            qk_chunk(0)
            for st in range(4):
                v_chunk(st)
            qk_chunk(1)
            for st in range(4, 8):
                v_chunk(st)
            qk_chunk(2)
            qk_chunk(3)
            for st in range(8, 16):
                v_chunk(st)

            # ---------------- attention -------------------------------------
            # scores transposed [kk, q]; pair p's two heads share a [128,1024]
            # psum tile. exp -> at (int16 tile, bf16 bit view). PV lags one
            # chunk: pair psum [128,512] = head 2p rows 0:64 (tile col 0),
            # head 2p+1 rows 64:128 (tile col 64); denominators = 4 concurrent
            # M=1 ones matmuls into psD partitions 0/32/64/96.
            exp_ctr = [0]

            def emit_scores_exp(m, kk):
                t = kk - 4 * m
                lo = max(0, t) * 128
                ksl = slice(128 * kk, 128 * (kk + 1))
                ats = []
                for p in range(2):
                    sps = psA.tile([128, 1024], fp32, tag="sc", name="sps")
                    for hh in range(2):
                        h = 2 * p + hh
                        hp = slice(32 * h, 32 * (h + 1))
                        tp = (32 * h, 0)
                        qsl = slice(512 * m + lo, 512 * (m + 1))
                        osl = slice(512 * hh + lo, 512 * hh + 512)
                        nc.tensor.matmul(sps[:, osl], k1[hp, ksl], q1[hp, qsl],
                                         start=True, stop=False, tile_position=tp)
                        nc.tensor.matmul(sps[:, osl], k2[hp, ksl], q2[hp, qsl],
                                         start=False, stop=True, tile_position=tp)
                    at = attn_pool.tile([128, 1024], i16, tag="attn", name="at")
                    at_bf = at[:, :].bitcast(bf16)
                    sps_v = sps[:, :].rearrange("a (h q) -> a h q", h=2)[:, :, lo:512]
                    at_v = at_bf.rearrange("a (h q) -> a h q", h=2)[:, :, lo:512]
                    use_dve = (
                        exp_mode == "dve"
                        or (exp_mode == "mix50" and exp_ctr[0] % 2 == 1)
                        or (exp_mode == "mix25" and exp_ctr[0] % 4 == 3)
                    )
                    exp_ctr[0] += 1
                    if use_dve:
                        at_i = at[:, :].rearrange("a (h q) -> a h q", h=2)[:, :, lo:512]
                        nc.vector.tensor_scalar(out=at_i, in0=sps_v,
                                                scalar1=EXP_A, scalar2=EXP_B,
                                                op0=MULT, op1=ADD)
                    else:
                        nc.scalar.activation(out=at_v, in_=sps_v, func=EXP, scale=0.125)
                    if t >= 0:
                        dv = at_bf.rearrange("a (h q) -> a h q", h=2)[:, :, 128 * t:128 * (t + 1)]
                        mv = maskt[:, :].rearrange("a (h q) -> a h q", h=2)
                        if mask_eng == "gps":
                            nc.gpsimd.tensor_tensor(dv, dv, mv, op=MULT)
                        else:
                            nc.vector.tensor_tensor(dv, dv, mv, op=MULT)
                    ats.append(at_bf)
                return ats, lo

            def emit_pv(m, kk, ats, lo, pv_ps, den_ps, last):
                for p in range(2):
                    for hh in range(2):
                        h = 2 * p + hh
                        nc.tensor.matmul(
                            pv_ps[p][64 * hh:64 * hh + 64, lo:512],
                            v_sb[kk][:, 64 * h:64 * h + 64],
                            ats[p][:, 512 * hh + lo:512 * hh + 512],
                            start=(kk == 0), stop=last)
                for h in range(HPC):
                    p, hh = h // 2, h % 2
                    nc.tensor.matmul(
                        den_ps[32 * h:32 * h + 1, lo:512],
                        ones1[:, 0:1],
                        ats[p][:, 512 * hh + lo:512 * hh + 512],
                        start=(kk == 0), stop=last,
                        tile_position=(0, 32 * h))

            def emit_normalize_a(m, pv_ps, den_ps):
                # PE-free part, emitted right at the macro boundary: free the
                # pv/den banks quickly, gather denominator rows, reciprocal.
                # DMA cannot read PSUM and engine APs cap base partition at
                # 64, so: engine-copy out, strided-DMA the scattered rows to
                # partition 0, reciprocal there.
                sp = [norm_pool.tile([128, 512], fp32, tag="sp", name="sp") for _ in range(2)]
                for p in range(2):
                    nc.vector.tensor_copy(sp[p], pv_ps[p])
                den_sb = norm_pool.tile([128, 512], fp32, tag="den_sb", name="den_sb")
                nc.scalar.copy(den_sb, den_ps)
                den4 = norm_pool.tile([4, 512], fp32, tag="den4", name="den4")
                den_v = den_sb[:, :].rearrange("(a b) w -> a b w", b=32)[:, 0, :]
                nc.sync.dma_start(out=den4, in_=den_v)
                if kdbg:
                    nc.sync.dma_start(out=dbg_den_d[:, 512 * m:512 * (m + 1)], in_=den4)
                rcp4 = norm_pool.tile([4, 512], fp32, tag="rcp4", name="rcp4")
                nc.vector.reciprocal_approx_fast(rcp4, den4)
                return sp, rcp4

            def emit_normalize_b(m, sp, rcp4):
                # PE part, deferred into the next macro's chunk stream so the
                # reciprocal chain latency never idles the PE at a boundary.
                # Broadcast reciprocal row h down 64 partitions with a K=4
                # selector matmul (sel4 col block 64h has row h all-ones);
                # partition_broadcast cannot write at a base-64 output.
                msl = slice(512 * m, 512 * (m + 1))
                for p in range(2):
                    rb_ps = psD.tile([128, 512], fp32, tag="rb", bufs=1, name="rb_ps")
                    nc.tensor.matmul(rb_ps[0:64, :], sel4[:, 64 * (2 * p):64 * (2 * p) + 64],
                                     rcp4, start=True, stop=True, tile_position=(0, 0))
                    nc.tensor.matmul(rb_ps[64:128, :], sel4[:, 64 * (2 * p + 1):64 * (2 * p + 1) + 64],
                                     rcp4, start=True, stop=True, tile_position=(0, 64))
                    nc.vector.tensor_tensor(aoT[p][:, msl], sp[p], rb_ps, op=MULT)

            def emit_outproj(m):
                msl = slice(512 * m, 512 * (m + 1))
                for fo in range(F_CH):
                    fsl = slice(128 * fo, 128 * (fo + 1))
                    pw = psA.tile([128, 512], fp32, tag="sc", name="pw")
                    for c in range(2):
                        nc.tensor.matmul(pw, wo[c][:, fsl], aoT[c][:, msl],
                                         start=(c == 0), stop=(c == 1))
                    ow = ostage.tile([128, 512], fp16, tag="ow", name="ow")
                    if fo % 2 == 0:
                        nc.vector.tensor_copy(ow, pw)
                    else:
                        nc.scalar.copy(ow, pw)
                    nc.sync.dma_start(out=outT_d[fsl, msl], in_=ow)

            # flat chunk stream across macros: scores at position i, PV at
            # i-2 (the lag carries across macro boundaries so the PE never
            # drains while an exp chain finishes), normalize/outproj of
            # macro m ride inside macro m+1's stream.
            chunks = [(m, kk) for m in range(MACROS) for kk in range(4 * m + 4)]
            LAG = 2
            pend = []       # (m, kk, ats, lo) awaiting PV
            actions = []    # deferred (countdown, fn) actions
            pv_ps = den_ps = None
            sp_rcp = {}

            def tick_actions():
                for a in list(actions):
                    a[0] -= 1
                    if a[0] <= 0:
                        actions.remove(a)
                        a[1]()

            def flush_one():
                nonlocal pv_ps, den_ps
                pm, pk, pats, plo = pend.pop(0)
                if pk == 0:
                    pv_ps = [psPV.tile([128, 512], fp32, tag="pvp", name="pvp") for _ in range(2)]
                    den_ps = psD.tile([128, 512], fp32, tag="den", bufs=1, name="den")
                last = pk == 4 * pm + 3
                emit_pv(pm, pk, pats, plo, pv_ps, den_ps, last=last)
                if last:
                    sp_rcp[pm] = emit_normalize_a(pm, pv_ps, den_ps)
                    actions.append([2, (lambda mm: lambda: emit_normalize_b(mm, *sp_rcp.pop(mm)))(pm)])
                    actions.append([4, (lambda mm: lambda: emit_outproj(mm))(pm)])

            for (m, kk) in chunks:
                ats, lo = emit_scores_exp(m, kk)
                if len(pend) >= LAG:
                    flush_one()
                pend.append([m, kk, ats, lo])
                tick_actions()
            while pend:
                flush_one()
                tick_actions()
            while actions:
                tick_actions()
            if kdbg:
                for i in range(2):
                    nc.sync.dma_start(out=dbg_ao_d[i][:, :], in_=aoT[i])
                nc.sync.dma_start(out=dbg_wo_d[:, :], in_=wo_all)

    nc.compile()
    return nc


def _get_nc():
    if "nc" not in _CACHE:
        _CACHE["nc"] = _build_nc()
    return _CACHE["nc"]


def _host_prep(x, positions, Wq, bq, Wk, bk, Wv, bv, Wo, bo):
    """Build the 8 per-core input maps."""
    ts = MAX_WAVELENGTH ** (2.0 * np.arange(HALF, dtype=np.float32) / D)  # [32]
    in_maps = []
    for c in range(NCORES):
        b, g = c // 4, c % 4
        heads = np.arange(4 * g, 4 * g + 4)
        cols_x1 = np.concatenate([64 * h + np.arange(32) for h in heads])
        cols_x2 = cols_x1 + 32
        perm = np.concatenate([cols_x1, cols_x2])

        # x^T quarter-chunk-major: block (n, kc) contiguous
        xTb = np.ascontiguousarray(x[b].T.astype(BF16))  # [F, S]
        xT = np.ascontiguousarray(
            xTb.reshape(F_CH, 128, N_CH, 512).transpose(2, 0, 1, 3)
        ).reshape(N_CH * F_CH * 128, 512)

        wv_e = np.zeros((F + 1, 256), dtype=np.float32)
        for hl, h in enumerate(heads):
            wv_e[:F, 64 * hl:64 * hl + 64] = Wv[:, 64 * h:64 * h + 64]
            wv_e[F, 64 * hl:64 * hl + 64] = bv[64 * h:64 * h + 64]

        bqk = np.stack([bq[cols_x1], bq[cols_x2], bk[cols_x1], bk[cols_x2]],
                       axis=1).astype(np.float32)  # [128, 4]

        pos = positions[b].astype(np.float32)  # [S]
        ang = pos[None, :] / ts[:, None]  # [32, S]
        cosw = np.tile(np.cos(ang), (4, 1)).astype(BF16)
        sinw = np.tile(np.sin(ang), (4, 1)).astype(BF16)

        ii = np.arange(128)
        mask = np.tile((ii[:, None] <= ii[None, :]).astype(BF16), (1, 2))

        sel4 = np.zeros((4, 256), dtype=np.float32)
        for h in range(4):
            sel4[h, 64 * h:64 * h + 64] = 1.0

        in_maps.append({
            "xT": xT,
            "wq": Wq[:, perm].astype(BF16),
            "wk": Wk[:, perm].astype(BF16),
            "wv": wv_e.astype(BF16),
            "wo": Wo[64 * heads[0]:64 * heads[0] + 256, :].astype(BF16),
            "bqk": bqk,
            "sel4": sel4,
            "cosw": cosw,
            "sinw": sinw,
            "mask": np.ascontiguousarray(mask),
        })
    return in_maps


def kernel(x, positions, Wq, bq, Wk, bk, Wv, bv, Wo, bo):
    global LAST_RESULT
    from concourse.bass_utils import run_bass_kernel_spmd

    x = np.asarray(x, dtype=np.float32)
    positions = np.asarray(positions)
    args = [np.asarray(a, dtype=np.float32) for a in (Wq, bq, Wk, bk, Wv, bv, Wo, bo)]
    Wq, bq, Wk, bk, Wv, bv, Wo, bo = args

    nc = _get_nc()
    in_maps = _host_prep(x, positions, Wq, bq, Wk, bk, Wv, bv, Wo, bo)
    try:
        res = run_bass_kernel_spmd(nc, in_maps, core_ids=list(range(NCORES)))
    except ModuleNotFoundError:
        # axon NTFF profiling hook unavailable in this image; run untraced
        os.environ["BASS_NEVER_TRACE"] = "1"
        res = run_bass_kernel_spmd(nc, in_maps, core_ids=list(range(NCORES)))
    LAST_RESULT = res

    out = np.empty((B, S, F), dtype=np.float32)
    for b in range(B):
        acc = np.zeros((F, S), dtype=np.float32)
        for g in range(4):
            acc += res.results[4 * b + g]["outT"].astype(np.float32)
        out[b] = acc.T + bo[None, :]
    return out
